# revision 11
# baseline (speedup 1.0000x reference)
"""Trainium2 Bass kernel for fused multi-tensor cosine-similarity loss.

Computes 1 - <r,d> / (|r| |d|) over 10 gradient tensors (5 rec + 5 data,
45,675,264 elements per side), data-parallel across 8 NeuronCores.

v4: bf16 storage + compute. The reduction is numerically forgiving
(measured output rel-err ~1e-7 in bf16 vs the 2e-2 gate), so inputs are
cast to bf16 on the host, halving HBM traffic per core from 45.7 MB to
22.8 MB.

Structure per core:
  - Host packs each side into 5 contiguous [128, 8192] bf16 tiles (2 MiB)
    plus 574/1024/2048-column tail chunks. DMA order is smallest-first so
    the first compute slice's data lands ~9 us in instead of ~18 us.
  - All input DMAs on the SP HWDGE ring; each DMA pair gets its own
    completion semaphore (a single running counter would assume cross-DMA
    completion ordering, which profiling perturbs - seen as NaNs).
  - Compute in [128, <=4096] slices. DVE runs scalar_tensor_tensor at 1x
    (~123 G elem/s; the 2x bf16 mode only exists for plain tensor_tensor,
    and InstTensorTensorReduce crashes NRT), ACT runs Square activations
    at ~158 G elem/s; the dot is DVE-only, squares are greedily split to
    balance the two engines (~61 us each).
  - Scratch outputs alternate between two SBUF buffers per engine (the
    accumulator columns in f32 are the real output).
  - Per-slice partials land as f32 columns of acc_v/acc_a, DMA'd out
    after each engine's drain; host reduces in float64 and applies the
    cosine combine. GpSimd resets sems afterwards for re-runnability.
"""

import sys

import ml_dtypes
import numpy as np

_REPO = "/opt/trn_rl_repo"
if _REPO not in sys.path:
    sys.path.insert(0, _REPO)

import concourse.bacc as bacc
import concourse.mybir as mybir
from concourse.bass_utils import run_bass_kernel_spmd

BF16 = ml_dtypes.bfloat16

C = 8  # cores
P = 128  # SBUF partitions
FD = 8192  # bf16 columns per full DMA tile (2 MiB)
FC = 4096  # max columns per compute slice
NBUF = 4  # full-tile double-buffer depth per side
TOTAL = 45_675_264  # elements per side
PER_CORE = TOTAL // C  # 5,709,408
COLS = 44_606  # ceil(PER_CORE/P) rounded up to even (160 pad elems)
NFULL = COLS // FD  # 5 full [P, FD] tiles
TAILS = [COLS - NFULL * FD - 3072, 1024, 2048]  # 574, 1024, 2048 (ascending)
assert NFULL * FD + sum(TAILS) == COLS
PADDED_PER_CORE = P * COLS
NTAIL = len(TAILS)
NPAIR = NTAIL + NFULL  # 8 DMA pairs, tails first


def _pair_width(p):
    return TAILS[p] if p < NTAIL else FD


# compute slices in DMA-pair order: one whole-pair op per reduction
# (coarse ops amortize the fixed per-op cost: 151 cyc DVE / 352 cyc ACT)
_SLICES = [(_p, 0, _pair_width(_p)) for _p in range(NPAIR)]
NSLICE = len(_SLICES)  # 8


def _dve_ns(w):
    # STT (TensorScalarPtr) runs at 1x: ~(w + 151) cycles @ 0.96 GHz
    return (w + 151) / 0.96


def _act_ns(w):
    return (w + 352) / 1.2


# Greedy square assignment balancing cumulative engine load in slice order.
# kind in {sp, rr, dd}; sp (the dot) is always DVE.
_V_OPS = []  # DVE program: (pair, off, w, kind)
_A_OPS = []  # ACT program
_v_load = 0.0
_a_load = 0.0
for _p, _off, _w in _SLICES:
    _V_OPS.append((_p, _off, _w, "sp"))
    _v_load += _dve_ns(_w)
    for _kind in ("rr", "dd"):
        if _v_load + _dve_ns(_w) <= _a_load + _act_ns(_w):
            _V_OPS.append((_p, _off, _w, _kind))
            _v_load += _dve_ns(_w)
        else:
            _A_OPS.append((_p, _off, _w, _kind))
            _a_load += _act_ns(_w)
NV = len(_V_OPS)
NA = len(_A_OPS)
V_KIND = [k for (_, _, _, k) in _V_OPS]
A_KIND = [k for (_, _, _, k) in _A_OPS]

# cumulative op counts once pair p is fully consumed (for WAR gating of
# the rotating full-tile buffers)
_NV_DONE = [0] * NPAIR
_NA_DONE = [0] * NPAIR
for _p, _off, _w, _k in _V_OPS:
    _NV_DONE[_p] += 1
for _p, _off, _w, _k in _A_OPS:
    _NA_DONE[_p] += 1
for _p in range(1, NPAIR):
    _NV_DONE[_p] += _NV_DONE[_p - 1]
    _NA_DONE[_p] += _NA_DONE[_p - 1]

_REC_KEYS = ("rec_emb", "rec_qkv", "rec_proj", "rec_fc1", "rec_fc2")
_DATA_KEYS = ("data_emb", "data_qkv", "data_proj", "data_fc1", "data_fc2")

_CACHE = {}


def _build():
    nc = bacc.Bacc("TRN2", target_bir_lowering=False, debug=False)
    f32 = mybir.dt.float32
    bf16 = mybir.dt.bfloat16
    r0 = nc.declare_dram_parameter("r0", [NFULL, P, FD], bf16, isOutput=False)
    d0 = nc.declare_dram_parameter("d0", [NFULL, P, FD], bf16, isOutput=False)
    rt = [
        nc.declare_dram_parameter(f"rt{j}", [P, w], bf16, isOutput=False)
        for j, w in enumerate(TAILS)
    ]
    dt = [
        nc.declare_dram_parameter(f"dt{j}", [P, w], bf16, isOutput=False)
        for j, w in enumerate(TAILS)
    ]
    o_v = nc.declare_dram_parameter("o_v", [P, NV], f32, isOutput=True)
    o_a = nc.declare_dram_parameter("o_a", [P, NA], f32, isOutput=True)

    rbuf = [nc.alloc_sbuf_tensor(f"rb{i}", [P, FD], bf16) for i in range(NBUF)]
    dbuf = [nc.alloc_sbuf_tensor(f"db{i}", [P, FD], bf16) for i in range(NBUF)]
    rtb = [nc.alloc_sbuf_tensor(f"rtb{j}", [P, w], bf16) for j, w in enumerate(TAILS)]
    dtb = [nc.alloc_sbuf_tensor(f"dtb{j}", [P, w], bf16) for j, w in enumerate(TAILS)]
    acc_v = nc.alloc_sbuf_tensor("acc_v", [P, NV], f32)
    acc_a = nc.alloc_sbuf_tensor("acc_a", [P, NA], f32)
    v_scr = nc.alloc_sbuf_tensor("v_scr", [P, FD], bf16)
    a_scr = nc.alloc_sbuf_tensor("a_scr", [P, FD], bf16)

    s_pair = [nc.alloc_semaphore(f"s_p{p}") for p in range(NPAIR)]
    s_v = nc.alloc_semaphore("s_v")
    s_a = nc.alloc_semaphore("s_a")
    s_out = nc.alloc_semaphore("s_out")
    _all_sems = s_pair + [s_v, s_a, s_out]
    sem_lo = min(s.num for s in _all_sems)
    sem_hi = max(s.num for s in _all_sems)
    sem_range = range(sem_lo, sem_hi + 1)

    def pair_bufs(p):
        if p < NTAIL:
            return rtb[p], dtb[p]
        return rbuf[(p - NTAIL) % NBUF], dbuf[(p - NTAIL) % NBUF]

    # ---- SP: all input DMAs in pair order, then the output DMAs ----
    for p in range(NPAIR):
        if p - NTAIL >= NBUF:
            # WAR: buffer reused from pair p-NBUF; wait out its readers
            nc.sync.wait_ge(s_v, _NV_DONE[p - NBUF])
            nc.sync.wait_ge(s_a, _NA_DONE[p - NBUF])
        rb, db = pair_bufs(p)
        if p < NTAIL:
            rsrc, dsrc = rt[p][:], dt[p][:]
        else:
            rsrc, dsrc = r0[p - NTAIL], d0[p - NTAIL]
        nc.sync.dma_start(out=rb[:, :], in_=rsrc).then_inc(s_pair[p], 16)
        nc.sync.dma_start(out=db[:, :], in_=dsrc).then_inc(s_pair[p], 16)
    # drain markers (+1) prove the accumulator writes retired
    nc.sync.wait_ge(s_v, NV + 1)
    nc.sync.dma_start(out=o_v[:], in_=acc_v[:]).then_inc(s_out, 16)
    nc.sync.wait_ge(s_a, NA + 1)
    nc.sync.dma_start(out=o_a[:], in_=acc_a[:]).then_inc(s_out, 16)

    # ---- GpSimd: wait for both outputs, then reset sems for re-runnability ----
    nc.gpsimd.wait_ge(s_out, 32)
    nc.gpsimd.dma_reset(sem_range)
    nc.gpsimd.sem_clear(sem_range)

    # ---- DVE: dot every slice + assigned squares ----
    nc_v_col = 0
    seen_v = -1
    for p, off, w, kind in _V_OPS:
        if p > seen_v:
            nc.vector.wait_ge(s_pair[p], 32)
            seen_v = p
        rb, db = pair_bufs(p)
        rs = rb[:, off : off + w]
        ds = db[:, off : off + w]
        in0, in1 = (rs, ds) if kind == "sp" else ((rs, rs) if kind == "rr" else (ds, ds))
        nc.vector.scalar_tensor_tensor(
            out=v_scr[:, :w],
            in0=in0,
            scalar=1.0,
            in1=in1,
            op0=mybir.AluOpType.bypass,
            op1=mybir.AluOpType.mult,
            accum_out=acc_v[:, nc_v_col : nc_v_col + 1],
        ).then_inc(s_v, 1)
        nc_v_col += 1
    nc.vector.drain()
    nc.vector.sem_inc(s_v, 1)

    # ---- ACT: remaining squares ----
    nc_a_col = 0
    seen_a = -1
    for p, off, w, kind in _A_OPS:
        if p > seen_a:
            nc.scalar.wait_ge(s_pair[p], 32)
            seen_a = p
        rb, db = pair_bufs(p)
        src = rb[:, off : off + w] if kind == "rr" else db[:, off : off + w]
        nc.scalar.activation(
            a_scr[:, :w],
            src,
            mybir.ActivationFunctionType.Square,
            accum_out=acc_a[:, nc_a_col : nc_a_col + 1],
        ).then_inc(s_a, 1)
        nc_a_col += 1
    nc.scalar.drain()
    nc.scalar.sem_inc(s_a, 1)

    nc.compile()
    return nc


def _get_nc():
    if "nc" not in _CACHE:
        _CACHE["nc"] = _build()
    return _CACHE["nc"]


def _pack(arrays):
    flat = np.concatenate(
        [np.asarray(a, dtype=np.float32).reshape(-1) for a in arrays]
    ).astype(BF16)
    assert flat.size == TOTAL
    buf = np.zeros((C, PADDED_PER_CORE), dtype=BF16)
    for c in range(C):
        buf[c, :PER_CORE] = flat[c * PER_CORE : (c + 1) * PER_CORE]
    # layout: tails first (ascending), then the 5 full tiles
    tails = []
    off = 0
    for w in TAILS:
        tails.append(buf[:, off : off + P * w].reshape(C, P, w))
        off += P * w
    main = buf[:, off:].reshape(C, NFULL, P, FD)
    return main, tails


def _run(inputs, trace=False, trace_cores=None):
    rmain, rtails = _pack([inputs[k] for k in _REC_KEYS])
    dmain, dtails = _pack([inputs[k] for k in _DATA_KEYS])
    in_maps = []
    for c in range(C):
        m = {"r0": rmain[c], "d0": dmain[c]}
        for j in range(NTAIL):
            m[f"rt{j}"] = rtails[j][c]
            m[f"dt{j}"] = dtails[j][c]
        in_maps.append(m)
    kwargs = {}
    if trace_cores is not None:
        kwargs["trace_cores"] = trace_cores
    res = run_bass_kernel_spmd(
        _get_nc(), in_maps, core_ids=list(range(C)), trace=trace, **kwargs
    )
    tot = {"sp": 0.0, "rr": 0.0, "dd": 0.0}
    for m in res.results:
        av = m["o_v"].astype(np.float64)
        aa = m["o_a"].astype(np.float64)
        for k, kind in enumerate(V_KIND):
            tot[kind] += av[:, k].sum()
        for k, kind in enumerate(A_KIND):
            tot[kind] += aa[:, k].sum()
    out = 1.0 - tot["sp"] / (np.sqrt(tot["rr"]) * np.sqrt(tot["dd"]))
    return np.array(out, dtype=np.float32), res


def kernel(**inputs):
    out, _ = _run(inputs, trace=False)
    return out


def kernel_traced(_trace_cores=None, **inputs):
    out, res = _run(inputs, trace=True, trace_cores=_trace_cores)
    return out, res
